# revision 5
# baseline (speedup 1.0000x reference)
"""Trainium2 Bass kernel for ContentBasedAttention (Bahdanau additive attention).

Reference math (per batch row b):
    enc_h  = enc_states @ W_enc + b_enc                  # [S, A]
    dec_h  = dec_states @ W_dec + b_dec                  # [A]
    score  = tanh(enc_h + dec_h) . w_attn                # [S]
    attn   = softmax(mask(score))                        # [S]
    ctx    = (attn @ enc_states) @ W_out + b_out         # [OUT]

Sharding: data-parallel over batch B=32 across 8 NeuronCores (4 rows/core).
Per core the kernel streams enc_states in 4 windows of 512 sequence
positions: SWDGE DMA casts fp32->bf16 on load (gpsimd queue), xbar
DMA-transpose (sync HWDGE ring, kept transpose-only) gives the
e-on-partitions layout the TensorEngine needs, weights load as fp32 on the
scalar HWDGE ring and cast to bf16 on the otherwise-idle VectorEngine.
tanh runs on ScalarE with the dec_h+biases folded in as the per-partition
activation bias; the w_attn dot and the attn@enc contraction run back on
the TensorEngine.  Softmax skips the max-subtraction (scores are bounded
by |w_attn|_1 ~ 16, exp can't overflow in fp32), so exp/mask/context
accumulate online per window and one reciprocal at the end of each row
normalizes both outputs.
"""

import sys

for _p in ("/opt/trn_rl_repo", "/root/.axon_site"):
    if _p not in sys.path:
        sys.path.insert(0, _p)

import numpy as np

import concourse.bass as bass
import concourse.mybir as mybir
from concourse import bacc
from concourse.bass import ds, ts
from concourse.bass_utils import run_bass_kernel_spmd
from concourse.masks import make_identity
from concourse.tile import TileContext

F32 = mybir.dt.float32
BF16 = mybir.dt.bfloat16
I32 = mybir.dt.int32
AF = mybir.ActivationFunctionType
ALU = mybir.AluOpType

N_CORES = 8
B, S, ENC_D, DEC_D, ATTN_D, OUT_D = 32, 2048, 1024, 1024, 1024, 1024
BL = B // N_CORES          # batch rows per core
P = 128
EC = ENC_D // P            # enc-dim chunks (contraction)
AT = ATTN_D // P           # attn-dim tiles
OT = OUT_D // P            # out-dim tiles
ST = S // P                # sequence tiles of 128
NW = 4                     # windows per batch row
WST = ST // NW             # sequence tiles per window (4)
WS = WST * P               # window size in sequence positions (512)


def build_kernel():
    nc = bacc.Bacc("TRN2", target_bir_lowering=False, debug=False,
                   num_devices=N_CORES)

    enc = nc.dram_tensor("enc", [BL, S, ENC_D], F32, kind="ExternalInput").ap()
    dec = nc.dram_tensor("dec", [BL, DEC_D], F32, kind="ExternalInput").ap()
    elen = nc.dram_tensor("elen", [1, BL], I32, kind="ExternalInput").ap()
    W_enc = nc.dram_tensor("W_enc", [ENC_D, ATTN_D], F32, kind="ExternalInput").ap()
    b_enc = nc.dram_tensor("b_enc", [ATTN_D], F32, kind="ExternalInput").ap()
    W_dec = nc.dram_tensor("W_dec", [DEC_D, ATTN_D], F32, kind="ExternalInput").ap()
    b_dec = nc.dram_tensor("b_dec", [ATTN_D], F32, kind="ExternalInput").ap()
    w_attn = nc.dram_tensor("w_attn", [ATTN_D], F32, kind="ExternalInput").ap()
    W_out = nc.dram_tensor("W_out", [ENC_D, OUT_D], F32, kind="ExternalInput").ap()
    b_out = nc.dram_tensor("b_out", [OUT_D], F32, kind="ExternalInput").ap()
    ctx_out = nc.dram_tensor("ctx_out", [BL, OUT_D], F32, kind="ExternalOutput").ap()
    attn_out = nc.dram_tensor("attn_out", [BL, S], F32, kind="ExternalOutput").ap()

    with TileContext(nc) as tc:
        with (
            tc.tile_pool(name="wpool", bufs=1) as wpool,
            tc.tile_pool(name="wtmp", bufs=2) as wtmp,
            tc.tile_pool(name="const", bufs=1) as constp,
            tc.tile_pool(name="nat", bufs=5) as natp,
            tc.tile_pool(name="trp", bufs=4) as trp,
            tc.tile_pool(name="tanh", bufs=3) as tanhp,
            tc.tile_pool(name="small", bufs=2) as smallp,
            tc.tile_pool(name="ph", bufs=2, space="PSUM") as psh,
            tc.tile_pool(name="psc", bufs=2, space="PSUM") as pssc,
            tc.tile_pool(name="pctx", bufs=2, space="PSUM") as psctx,
            tc.tile_pool(name="pmisc", bufs=2, space="PSUM") as psmisc,
        ):
            # ---- first enc window starts streaming immediately (SWDGE queue
            # carries only these big cast-loads) ----
            nats = {}
            trs = {}

            def load_window(b, w):
                enc_b = enc[b].rearrange("(st p) e -> p st e", p=P)
                nat = natp.tile([P, WST, ENC_D], BF16, tag="nat")
                nc.gpsimd.dma_start(nat[:], enc_b[:, ds(w * WST, WST), :])
                tr = trp.tile([P, WST, EC, P], BF16, tag="encT")
                for st in range(WST):
                    nc.sync.dma_start_transpose(tr[:, st], nat[:, st, :])
                nats[(b, w)] = nat
                trs[(b, w)] = tr

            load_window(0, 0)

            # ---- small constants on the scalar HWDGE ring ----
            benc_sb = constp.tile([P, AT], F32, tag="benc")
            nc.scalar.dma_start(benc_sb[:], b_enc.rearrange("(t p) -> p t", p=P))
            bdec_sb = constp.tile([P, AT], F32, tag="bdec")
            nc.scalar.dma_start(bdec_sb[:], b_dec.rearrange("(t p) -> p t", p=P))
            bsum_sb = constp.tile([P, AT], F32, tag="bsum")
            nc.vector.tensor_tensor(bsum_sb[:], benc_sb[:], bdec_sb[:], op=ALU.add)
            wattn_f = constp.tile([P, AT], F32, tag="wattn_f")
            nc.scalar.dma_start(wattn_f[:], w_attn.rearrange("(c p) -> p c", p=P))
            wattn_sb = constp.tile([P, AT], BF16, tag="wattn")
            nc.vector.tensor_copy(wattn_sb[:], wattn_f[:])
            dec_sb = constp.tile([BL, DEC_D], F32, tag="dec_sb")
            nc.scalar.dma_start(dec_sb[:], dec)
            dec_bf = constp.tile([BL, DEC_D], BF16, tag="dec_bf")
            nc.vector.tensor_copy(dec_bf[:], dec_sb[:])
            elen_i = constp.tile([1, BL], I32, tag="elen_i")
            nc.scalar.dma_start(elen_i[:], elen)
            elen_f = constp.tile([1, BL], F32, tag="elen_f")
            nc.vector.tensor_copy(elen_f[:], elen_i[:])

            iota_i = constp.tile([P, ST], I32, tag="iota_i")
            nc.gpsimd.iota(iota_i[:], pattern=[[P, ST]], base=0, channel_multiplier=1)
            iota_f = constp.tile([P, ST], F32, tag="iota_f")
            nc.vector.tensor_copy(iota_f[:], iota_i[:])

            ident = constp.tile([P, P], BF16, tag="ident")
            make_identity(nc, ident[:])
            ones_col = constp.tile([P, 1], F32, tag="ones_col")
            nc.vector.memset(ones_col[:], 1.0)
            ones_row = constp.tile([1, P], F32, tag="ones_row")
            nc.vector.memset(ones_row[:], 1.0)

            elen_ps = psmisc.tile([P, BL], F32, tag="misc")
            nc.tensor.matmul(elen_ps[:], ones_row[:], elen_f[:], start=True, stop=True)
            len_bc = constp.tile([P, BL], F32, tag="len_bc")
            nc.vector.tensor_copy(len_bc[:], elen_ps[:])

            # ---- weights: fp32 on the scalar ring, bf16 cast on VectorE,
            # sliced per a-tile so the first matmul can start early ----
            def load_w(dst_sb, src):
                src3 = src.rearrange("(c p) a -> p c a", p=P)
                for t in range(AT):
                    tmp = wtmp.tile([P, EC, P], F32, tag="wtmp")
                    nc.scalar.dma_start(tmp[:], src3[:, :, ts(t, P)])
                    nc.vector.tensor_copy(dst_sb[:, :, ts(t, P)], tmp[:])

            Wenc_sb = wpool.tile([P, EC, ATTN_D], BF16, tag="Wenc")
            load_w(Wenc_sb, W_enc)
            load_window(0, 1)
            Wdec_sb = wpool.tile([P, EC, ATTN_D], BF16, tag="Wdec")
            load_w(Wdec_sb, W_dec)

            # ---- dec path: combined per-(a,b) tanh bias = dec@W_dec+b_dec+b_enc
            decT = constp.tile([P, EC, BL], BF16, tag="decT")
            for c in range(EC):
                tp = psmisc.tile([P, BL], BF16, tag="misc")
                nc.tensor.transpose(tp[:], dec_bf[:, ts(c, P)], ident[0:BL, 0:BL])
                nc.vector.tensor_copy(decT[:, c, :], tp[:])
            cbias = constp.tile([P, AT, BL], F32, tag="cbias")
            for t in range(AT):
                hp = psmisc.tile([P, BL], F32, tag="misc")
                for c in range(EC):
                    nc.tensor.matmul(hp[:], Wdec_sb[:, c, ts(t, P)], decT[:, c, :],
                                     start=(c == 0), stop=(c == EC - 1))
                nc.scalar.activation(cbias[:, t, :], hp[:], AF.Identity,
                                     bias=bsum_sb[:, t:t + 1], scale=1.0)

            load_window(0, 2)
            Wout_sb = wpool.tile([P, EC, OUT_D], BF16, tag="Wout")
            load_w(Wout_sb, W_out)
            bout_sb = constp.tile([P, OT], F32, tag="bout")
            nc.scalar.dma_start(bout_sb[:], b_out.rearrange("(t p) -> p t", p=P))

            ctxT_all = constp.tile([P, EC, BL], BF16, tag="ctxT_all")

            # ---- main loop over local batch rows ----
            for b in range(BL):
                scores_ps = pssc.tile([P, ST], F32, tag="scores")
                expm = smallp.tile([P, ST], F32, tag="expm")
                expm_bf = smallp.tile([P, ST], BF16, tag="expm_bf")
                ctx_acc = smallp.tile([P, EC], F32, tag="ctx_acc")

                for w in range(NW):
                    if (b, w) not in nats:
                        load_window(b, w)
                    # prefetch next window
                    nb, nw = (b, w + 1) if w + 1 < NW else (b + 1, 0)
                    if nb < BL and (nb, nw) not in nats:
                        load_window(nb, nw)

                    nat = nats[(b, w)]
                    tr = trs[(b, w)]

                    th = tanhp.tile([P, AT, WS], BF16, tag="tanh")
                    for t in range(AT):
                        hp = psh.tile([P, WS], F32, tag="hmain")
                        for c in range(EC):
                            nc.tensor.matmul(hp[:], Wenc_sb[:, c, ts(t, P)],
                                             tr[:, :, c, :],
                                             start=(c == 0), stop=(c == EC - 1))
                        nc.scalar.activation(th[:, t, :], hp[:], AF.Tanh,
                                             bias=cbias[:, t, b:b + 1], scale=1.0)

                    # scores for the 4 sequence tiles of this window
                    for j in range(WST):
                        sj = w * WST + j
                        for t in range(AT):
                            nc.tensor.matmul(scores_ps[:, sj:sj + 1],
                                             th[:, t, ts(j, P)],
                                             wattn_sb[:, t:t + 1],
                                             start=(t == 0), stop=(t == AT - 1))

                    # masked exp for this window (no max-sub: scores bounded)
                    ex = smallp.tile([P, WST], F32, tag="ex")
                    nc.scalar.activation(ex[:], scores_ps[:, ds(w * WST, WST)], AF.Exp)
                    va = smallp.tile([P, WST], F32, tag="va")
                    nc.vector.tensor_scalar(va[:], iota_f[:, ds(w * WST, WST)],
                                            len_bc[:, b:b + 1], None, op0=ALU.is_lt)
                    nc.vector.tensor_tensor(expm[:, ds(w * WST, WST)], ex[:], va[:],
                                            op=ALU.mult)
                    nc.vector.tensor_copy(expm_bf[:, ds(w * WST, WST)],
                                          expm[:, ds(w * WST, WST)])

                    # unnormalized context partial: enc_nat^T @ expm
                    cp = psctx.tile([P, EC], F32, tag="cp")
                    for t in range(EC):
                        for st in range(WST):
                            nc.tensor.matmul(cp[:, t:t + 1], nat[:, st, ts(t, P)],
                                             expm_bf[:, w * WST + st:w * WST + st + 1],
                                             start=(st == 0), stop=(st == WST - 1))
                    if w == 0:
                        nc.vector.tensor_copy(ctx_acc[:], cp[:])
                    else:
                        nc.vector.tensor_tensor(ctx_acc[:], ctx_acc[:], cp[:],
                                                op=ALU.add)
                    del nats[(b, w)], trs[(b, w)]

                # softmax denominator: cross-partition sum via PE ones-matmul
                sums = smallp.tile([P, 1], F32, tag="sums")
                nc.vector.tensor_reduce(sums[:], expm[:], axis=mybir.AxisListType.X,
                                        op=ALU.add)
                tot_ps = psmisc.tile([1, 1], F32, tag="misc")
                nc.tensor.matmul(tot_ps[:], ones_col[:], sums[:], start=True, stop=True)
                recip = smallp.tile([1, 1], F32, tag="recip")
                nc.vector.reciprocal(recip[:], tot_ps[:])
                rb_ps = psmisc.tile([P, 1], F32, tag="misc")
                nc.tensor.matmul(rb_ps[:], ones_row[:], recip[:], start=True, stop=True)
                rb = smallp.tile([P, 1], F32, tag="rb")
                nc.vector.tensor_copy(rb[:], rb_ps[:])

                attn_f = smallp.tile([P, ST], F32, tag="attn_f")
                nc.vector.tensor_scalar(attn_f[:], expm[:], rb[:], None, op0=ALU.mult)
                nc.scalar.dma_start(attn_out[b].rearrange("(j p) -> p j", p=P),
                                    attn_f[:])
                nc.vector.tensor_scalar(ctxT_all[:, :, b], ctx_acc[:], rb[:], None,
                                        op0=ALU.mult)

            # ---- final projection: ctx @ W_out + b_out ----
            out_sb = constp.tile([P, OT, BL], F32, tag="out_sb")
            for t in range(OT):
                op_ps = psmisc.tile([P, BL], F32, tag="misc")
                for c in range(EC):
                    nc.tensor.matmul(op_ps[:], Wout_sb[:, c, ts(t, P)],
                                     ctxT_all[:, c, :],
                                     start=(c == 0), stop=(c == EC - 1))
                nc.scalar.activation(out_sb[:, t, :], op_ps[:], AF.Identity,
                                     bias=bout_sb[:, t:t + 1], scale=1.0)
            for b in range(BL):
                nc.scalar.dma_start(ctx_out[b].rearrange("(t p) -> p t", p=P),
                                    out_sb[:, :, b])

    nc.compile()
    return nc


_NC_CACHE = {}


def _get_nc():
    if "nc" not in _NC_CACHE:
        _NC_CACHE["nc"] = build_kernel()
    return _NC_CACHE["nc"]


def kernel(enc_states, dec_states, W_enc, b_enc, W_dec, b_dec, w_attn, W_out,
           b_out, enc_len, _trace=False, _trace_kwargs=None):
    nc = _get_nc()

    enc_states = np.ascontiguousarray(np.asarray(enc_states, dtype=np.float32))
    dec_states = np.ascontiguousarray(np.asarray(dec_states, dtype=np.float32))
    enc_len_i = np.asarray(enc_len).astype(np.int32)
    shared = {
        "W_enc": np.ascontiguousarray(np.asarray(W_enc, dtype=np.float32)),
        "b_enc": np.ascontiguousarray(np.asarray(b_enc, dtype=np.float32)),
        "W_dec": np.ascontiguousarray(np.asarray(W_dec, dtype=np.float32)),
        "b_dec": np.ascontiguousarray(np.asarray(b_dec, dtype=np.float32)),
        "w_attn": np.ascontiguousarray(np.asarray(w_attn, dtype=np.float32)),
        "W_out": np.ascontiguousarray(np.asarray(W_out, dtype=np.float32)),
        "b_out": np.ascontiguousarray(np.asarray(b_out, dtype=np.float32)),
    }
    in_maps = []
    for i in range(N_CORES):
        sl = slice(i * BL, (i + 1) * BL)
        in_maps.append({
            "enc": enc_states[sl],
            "dec": dec_states[sl],
            "elen": enc_len_i[sl].reshape(1, BL),
            **shared,
        })

    res = run_bass_kernel_spmd(nc, in_maps, core_ids=list(range(N_CORES)),
                               trace=_trace, **(_trace_kwargs or {}))

    context = np.concatenate([r["ctx_out"] for r in res.results], axis=0)
    attn = np.concatenate([r["attn_out"] for r in res.results], axis=0)
    if _trace:
        return (context, attn), res
    return (context, attn)


# revision 6
# speedup vs baseline: 1.2215x; 1.2215x over previous
"""Trainium2 Bass kernel for ContentBasedAttention (Bahdanau additive attention).

Reference math (per batch row b):
    enc_h  = enc_states @ W_enc + b_enc                  # [S, A]
    dec_h  = dec_states @ W_dec + b_dec                  # [A]
    score  = tanh(enc_h + dec_h) . w_attn                # [S]
    attn   = softmax(mask(score))                        # [S]
    ctx    = (attn @ enc_states) @ W_out + b_out         # [OUT]

Sharding: data-parallel over batch B=32 across 8 NeuronCores (4 rows/core).
Per core the kernel streams enc_states in 4 windows of 512 sequence
positions: SWDGE DMA casts fp32->bf16 on load (gpsimd queue), xbar
DMA-transpose (sync HWDGE ring, kept transpose-only) gives the
e-on-partitions layout the TensorEngine needs, weights load as fp32 on the
scalar HWDGE ring and cast to bf16 on the otherwise-idle VectorEngine.
tanh runs on ScalarE with the dec_h+biases folded in as the per-partition
activation bias; the w_attn dot and the attn@enc contraction run back on
the TensorEngine.  Softmax skips the max-subtraction (scores are bounded
by |w_attn|_1 ~ 16, exp can't overflow in fp32), so exp/mask/context
accumulate online per window and one reciprocal at the end of each row
normalizes both outputs.
"""

import sys

for _p in ("/opt/trn_rl_repo", "/root/.axon_site"):
    if _p not in sys.path:
        sys.path.insert(0, _p)

import numpy as np

import concourse.bass as bass
import concourse.mybir as mybir
from concourse import bacc
from concourse.bass import ds, ts
from concourse.bass_utils import run_bass_kernel_spmd
from concourse.masks import make_identity
from concourse.tile import TileContext

F32 = mybir.dt.float32
BF16 = mybir.dt.bfloat16
I32 = mybir.dt.int32
AF = mybir.ActivationFunctionType
ALU = mybir.AluOpType

N_CORES = 8
B, S, ENC_D, DEC_D, ATTN_D, OUT_D = 32, 2048, 1024, 1024, 1024, 1024
BL = B // N_CORES          # batch rows per core
P = 128
EC = ENC_D // P            # enc-dim chunks (contraction)
AT = ATTN_D // P           # attn-dim tiles
OT = OUT_D // P            # out-dim tiles
ST = S // P                # sequence tiles of 128
NW = 4                     # windows per batch row
WST = ST // NW             # sequence tiles per window (4)
WS = WST * P               # window size in sequence positions (512)


def build_kernel():
    nc = bacc.Bacc("TRN2", target_bir_lowering=False, debug=False,
                   num_devices=N_CORES)

    enc = nc.dram_tensor("enc", [BL, S, ENC_D], F32, kind="ExternalInput").ap()
    dec = nc.dram_tensor("dec", [BL, DEC_D], F32, kind="ExternalInput").ap()
    elen = nc.dram_tensor("elen", [1, BL], I32, kind="ExternalInput").ap()
    W_enc = nc.dram_tensor("W_enc", [ENC_D, ATTN_D], F32, kind="ExternalInput").ap()
    b_enc = nc.dram_tensor("b_enc", [ATTN_D], F32, kind="ExternalInput").ap()
    W_dec = nc.dram_tensor("W_dec", [DEC_D, ATTN_D], F32, kind="ExternalInput").ap()
    b_dec = nc.dram_tensor("b_dec", [ATTN_D], F32, kind="ExternalInput").ap()
    w_attn = nc.dram_tensor("w_attn", [ATTN_D], F32, kind="ExternalInput").ap()
    W_out = nc.dram_tensor("W_out", [ENC_D, OUT_D], F32, kind="ExternalInput").ap()
    b_out = nc.dram_tensor("b_out", [OUT_D], F32, kind="ExternalInput").ap()
    ctx_out = nc.dram_tensor("ctx_out", [BL, OUT_D], F32, kind="ExternalOutput").ap()
    attn_out = nc.dram_tensor("attn_out", [BL, S], F32, kind="ExternalOutput").ap()

    with TileContext(nc) as tc:
        with (
            tc.tile_pool(name="wpool", bufs=1) as wpool,
            tc.tile_pool(name="wtmp", bufs=3) as wtmp,
            tc.tile_pool(name="const", bufs=1) as constp,
            tc.tile_pool(name="nat", bufs=6) as natp,
            tc.tile_pool(name="trp", bufs=5) as trp,
            tc.tile_pool(name="tanh", bufs=2) as tanhp,
            tc.tile_pool(name="small", bufs=2) as smallp,
            tc.tile_pool(name="ph", bufs=3, space="PSUM") as psh,
            tc.tile_pool(name="psc", bufs=2, space="PSUM") as pssc,
            tc.tile_pool(name="pctx", bufs=2, space="PSUM") as psctx,
            tc.tile_pool(name="pmisc", bufs=1, space="PSUM") as psmisc,
        ):
            # ---- first enc window starts streaming immediately (SWDGE queue
            # carries only these big cast-loads) ----
            nats = {}
            trs = {}

            def load_window(b, w):
                enc_b = enc[b].rearrange("(st p) e -> p st e", p=P)
                nat = natp.tile([P, WST, ENC_D], BF16, tag="nat")
                nc.gpsimd.dma_start(nat[:], enc_b[:, ds(w * WST, WST), :])
                tr = trp.tile([P, WST, EC, P], BF16, tag="encT")
                for st in range(WST):
                    nc.sync.dma_start_transpose(tr[:, st], nat[:, st, :])
                nats[(b, w)] = nat
                trs[(b, w)] = tr

            load_window(0, 0)

            # ---- small constants on the scalar HWDGE ring ----
            benc_sb = constp.tile([P, AT], F32, tag="benc")
            nc.scalar.dma_start(benc_sb[:], b_enc.rearrange("(t p) -> p t", p=P))
            bdec_sb = constp.tile([P, AT], F32, tag="bdec")
            nc.scalar.dma_start(bdec_sb[:], b_dec.rearrange("(t p) -> p t", p=P))
            bsum_sb = constp.tile([P, AT], F32, tag="bsum")
            nc.vector.tensor_tensor(bsum_sb[:], benc_sb[:], bdec_sb[:], op=ALU.add)
            wattn_f = constp.tile([P, AT], F32, tag="wattn_f")
            nc.scalar.dma_start(wattn_f[:], w_attn.rearrange("(c p) -> p c", p=P))
            wattn_sb = constp.tile([P, AT], BF16, tag="wattn")
            nc.vector.tensor_copy(wattn_sb[:], wattn_f[:])
            dec_sb = constp.tile([BL, DEC_D], F32, tag="dec_sb")
            nc.scalar.dma_start(dec_sb[:], dec)
            dec_bf = constp.tile([BL, DEC_D], BF16, tag="dec_bf")
            nc.vector.tensor_copy(dec_bf[:], dec_sb[:])
            elen_i = constp.tile([1, BL], I32, tag="elen_i")
            nc.scalar.dma_start(elen_i[:], elen)
            elen_f = constp.tile([1, BL], F32, tag="elen_f")
            nc.vector.tensor_copy(elen_f[:], elen_i[:])

            iota_i = constp.tile([P, ST], I32, tag="iota_i")
            nc.gpsimd.iota(iota_i[:], pattern=[[P, ST]], base=0, channel_multiplier=1)
            iota_f = constp.tile([P, ST], F32, tag="iota_f")
            nc.vector.tensor_copy(iota_f[:], iota_i[:])

            ident = constp.tile([P, P], BF16, tag="ident")
            make_identity(nc, ident[:])
            ones_col = constp.tile([P, 1], F32, tag="ones_col")
            nc.vector.memset(ones_col[:], 1.0)
            ones_row = constp.tile([1, P], F32, tag="ones_row")
            nc.vector.memset(ones_row[:], 1.0)

            elen_ps = psmisc.tile([P, BL], F32, tag="misc")
            nc.tensor.matmul(elen_ps[:], ones_row[:], elen_f[:], start=True, stop=True)
            len_bc = constp.tile([P, BL], F32, tag="len_bc")
            nc.vector.tensor_copy(len_bc[:], elen_ps[:])

            # ---- weights. Wdec goes over the scalar HWDGE ring (fp32) with
            # VectorE casting to bf16 so cbias is ready early; Wenc streams as
            # per-a-tile SWDGE cast-DMAs right behind the first window so the
            # first matmul can start ~9us in; Wout is deferred (needed last).
            Wdec_sb = wpool.tile([P, EC, ATTN_D], BF16, tag="Wdec")
            Wdec3 = W_dec.rearrange("(c p) a -> p c a", p=P)
            for t in range(AT):
                tmp = wtmp.tile([P, EC, P], F32, tag="wtmp")
                nc.scalar.dma_start(tmp[:], Wdec3[:, :, ts(t, P)])
                nc.vector.tensor_copy(Wdec_sb[:, :, ts(t, P)], tmp[:])

            Wenc_sb = wpool.tile([P, EC, ATTN_D], BF16, tag="Wenc")
            Wenc3 = W_enc.rearrange("(c p) a -> p c a", p=P)
            for t in range(AT):
                nc.gpsimd.dma_start(Wenc_sb[:, :, ts(t, P)], Wenc3[:, :, ts(t, P)])
            load_window(0, 1)

            # ---- dec path: combined per-(a,b) tanh bias = dec@W_dec+b_dec+b_enc
            decT = constp.tile([P, EC, BL], BF16, tag="decT")
            for c in range(EC):
                tp = psmisc.tile([P, BL], BF16, tag="misc")
                nc.tensor.transpose(tp[:], dec_bf[:, ts(c, P)], ident[0:BL, 0:BL])
                nc.vector.tensor_copy(decT[:, c, :], tp[:])
            cbias = constp.tile([P, AT, BL], F32, tag="cbias")
            for t in range(AT):
                hp = psmisc.tile([P, BL], F32, tag="misc")
                for c in range(EC):
                    nc.tensor.matmul(hp[:], Wdec_sb[:, c, ts(t, P)], decT[:, c, :],
                                     start=(c == 0), stop=(c == EC - 1))
                nc.scalar.activation(cbias[:, t, :], hp[:], AF.Identity,
                                     bias=bsum_sb[:, t:t + 1], scale=1.0)

            load_window(0, 2)
            Wout_sb = wpool.tile([P, EC, OUT_D], BF16, tag="Wout")
            bout_sb = constp.tile([P, OT], F32, tag="bout")
            nc.scalar.dma_start(bout_sb[:], b_out.rearrange("(t p) -> p t", p=P))

            ctxT_all = constp.tile([P, EC, BL], BF16, tag="ctxT_all")

            # ---- main loop over local batch rows ----
            for b in range(BL):
                scores_ps = pssc.tile([P, ST], F32, tag="scores")
                expm = smallp.tile([P, ST], F32, tag="expm")
                expm_bf = smallp.tile([P, ST], BF16, tag="expm_bf")
                ctx_acc = smallp.tile([P, EC], F32, tag="ctx_acc")

                for w in range(NW):
                    if (b, w) not in nats:
                        load_window(b, w)
                    # prefetch two windows ahead
                    gi = b * NW + w
                    for gj in (gi + 1, gi + 2):
                        nb, nw = divmod(gj, NW)
                        if nb < BL and (nb, nw) not in nats:
                            load_window(nb, nw)
                    if b == BL - 1 and w == 2:
                        # W_out is only needed by the tail projection; queue its
                        # cast-load behind the last enc windows
                        nc.gpsimd.dma_start(
                            Wout_sb[:], W_out.rearrange("(c p) a -> p c a", p=P))

                    nat = nats[(b, w)]
                    tr = trs[(b, w)]

                    th = tanhp.tile([P, AT, WS], BF16, tag="tanh")
                    for t in range(AT):
                        hp = psh.tile([P, WS], F32, tag="hmain")
                        for c in range(EC):
                            nc.tensor.matmul(hp[:], Wenc_sb[:, c, ts(t, P)],
                                             tr[:, :, c, :],
                                             start=(c == 0), stop=(c == EC - 1))
                        nc.scalar.activation(th[:, t, :], hp[:], AF.Tanh,
                                             bias=cbias[:, t, b:b + 1], scale=1.0)

                    # scores for the 4 sequence tiles of this window
                    for j in range(WST):
                        sj = w * WST + j
                        for t in range(AT):
                            nc.tensor.matmul(scores_ps[:, sj:sj + 1],
                                             th[:, t, ts(j, P)],
                                             wattn_sb[:, t:t + 1],
                                             start=(t == 0), stop=(t == AT - 1))

                    # masked exp for this window (no max-sub: scores bounded)
                    ex = smallp.tile([P, WST], F32, tag="ex")
                    nc.scalar.activation(ex[:], scores_ps[:, ds(w * WST, WST)], AF.Exp)
                    va = smallp.tile([P, WST], F32, tag="va")
                    nc.vector.tensor_scalar(va[:], iota_f[:, ds(w * WST, WST)],
                                            len_bc[:, b:b + 1], None, op0=ALU.is_lt)
                    nc.vector.tensor_tensor(expm[:, ds(w * WST, WST)], ex[:], va[:],
                                            op=ALU.mult)
                    nc.vector.tensor_copy(expm_bf[:, ds(w * WST, WST)],
                                          expm[:, ds(w * WST, WST)])

                    # unnormalized context partial: enc_nat^T @ expm
                    cp = psctx.tile([P, EC], F32, tag="cp")
                    for t in range(EC):
                        for st in range(WST):
                            nc.tensor.matmul(cp[:, t:t + 1], nat[:, st, ts(t, P)],
                                             expm_bf[:, w * WST + st:w * WST + st + 1],
                                             start=(st == 0), stop=(st == WST - 1))
                    if w == 0:
                        nc.vector.tensor_copy(ctx_acc[:], cp[:])
                    else:
                        nc.vector.tensor_tensor(ctx_acc[:], ctx_acc[:], cp[:],
                                                op=ALU.add)
                    del nats[(b, w)], trs[(b, w)]

                # softmax denominator: cross-partition sum via PE ones-matmul
                sums = smallp.tile([P, 1], F32, tag="sums")
                nc.vector.tensor_reduce(sums[:], expm[:], axis=mybir.AxisListType.X,
                                        op=ALU.add)
                tot_ps = psmisc.tile([1, 1], F32, tag="misc")
                nc.tensor.matmul(tot_ps[:], ones_col[:], sums[:], start=True, stop=True)
                recip = smallp.tile([1, 1], F32, tag="recip")
                nc.vector.reciprocal(recip[:], tot_ps[:])
                rb_ps = psmisc.tile([P, 1], F32, tag="misc")
                nc.tensor.matmul(rb_ps[:], ones_row[:], recip[:], start=True, stop=True)
                rb = smallp.tile([P, 1], F32, tag="rb")
                nc.vector.tensor_copy(rb[:], rb_ps[:])

                attn_f = smallp.tile([P, ST], F32, tag="attn_f")
                nc.vector.tensor_scalar(attn_f[:], expm[:], rb[:], None, op0=ALU.mult)
                nc.scalar.dma_start(attn_out[b].rearrange("(j p) -> p j", p=P),
                                    attn_f[:])
                nc.vector.tensor_scalar(ctxT_all[:, :, b], ctx_acc[:], rb[:], None,
                                        op0=ALU.mult)

            # ---- final projection: ctx @ W_out + b_out ----
            out_sb = constp.tile([P, OT, BL], F32, tag="out_sb")
            for t in range(OT):
                op_ps = psmisc.tile([P, BL], F32, tag="misc")
                for c in range(EC):
                    nc.tensor.matmul(op_ps[:], Wout_sb[:, c, ts(t, P)],
                                     ctxT_all[:, c, :],
                                     start=(c == 0), stop=(c == EC - 1))
                nc.scalar.activation(out_sb[:, t, :], op_ps[:], AF.Identity,
                                     bias=bout_sb[:, t:t + 1], scale=1.0)
            for b in range(BL):
                nc.scalar.dma_start(ctx_out[b].rearrange("(t p) -> p t", p=P),
                                    out_sb[:, :, b])

    nc.compile()
    return nc


_NC_CACHE = {}


def _get_nc():
    if "nc" not in _NC_CACHE:
        _NC_CACHE["nc"] = build_kernel()
    return _NC_CACHE["nc"]


def kernel(enc_states, dec_states, W_enc, b_enc, W_dec, b_dec, w_attn, W_out,
           b_out, enc_len, _trace=False, _trace_kwargs=None):
    nc = _get_nc()

    enc_states = np.ascontiguousarray(np.asarray(enc_states, dtype=np.float32))
    dec_states = np.ascontiguousarray(np.asarray(dec_states, dtype=np.float32))
    enc_len_i = np.asarray(enc_len).astype(np.int32)
    shared = {
        "W_enc": np.ascontiguousarray(np.asarray(W_enc, dtype=np.float32)),
        "b_enc": np.ascontiguousarray(np.asarray(b_enc, dtype=np.float32)),
        "W_dec": np.ascontiguousarray(np.asarray(W_dec, dtype=np.float32)),
        "b_dec": np.ascontiguousarray(np.asarray(b_dec, dtype=np.float32)),
        "w_attn": np.ascontiguousarray(np.asarray(w_attn, dtype=np.float32)),
        "W_out": np.ascontiguousarray(np.asarray(W_out, dtype=np.float32)),
        "b_out": np.ascontiguousarray(np.asarray(b_out, dtype=np.float32)),
    }
    in_maps = []
    for i in range(N_CORES):
        sl = slice(i * BL, (i + 1) * BL)
        in_maps.append({
            "enc": enc_states[sl],
            "dec": dec_states[sl],
            "elen": enc_len_i[sl].reshape(1, BL),
            **shared,
        })

    res = run_bass_kernel_spmd(nc, in_maps, core_ids=list(range(N_CORES)),
                               trace=_trace, **(_trace_kwargs or {}))

    context = np.concatenate([r["ctx_out"] for r in res.results], axis=0)
    attn = np.concatenate([r["attn_out"] for r in res.results], axis=0)
    if _trace:
        return (context, attn), res
    return (context, attn)


# revision 7
# speedup vs baseline: 1.3617x; 1.1147x over previous
"""Trainium2 Bass kernel for ContentBasedAttention (Bahdanau additive attention).

Reference math (per batch row b):
    enc_h  = enc_states @ W_enc + b_enc                  # [S, A]
    dec_h  = dec_states @ W_dec + b_dec                  # [A]
    score  = tanh(enc_h + dec_h) . w_attn                # [S]
    attn   = softmax(mask(score))                        # [S]
    ctx    = (attn @ enc_states) @ W_out + b_out         # [OUT]

Sharding: data-parallel over batch B=32 across 8 NeuronCores (4 rows/core).
Per core the kernel streams enc_states in 4 windows of 512 sequence
positions: SWDGE DMA casts fp32->bf16 on load (gpsimd queue), xbar
DMA-transpose (sync HWDGE ring, kept transpose-only) gives the
e-on-partitions layout the TensorEngine needs, weights load as fp32 on the
scalar HWDGE ring and cast to bf16 on the otherwise-idle VectorEngine.
tanh runs on ScalarE with the dec_h+biases folded in as the per-partition
activation bias; the w_attn dot and the attn@enc contraction run back on
the TensorEngine.  Softmax skips the max-subtraction (scores are bounded
by |w_attn|_1 ~ 16, exp can't overflow in fp32), so exp/mask/context
accumulate online per window and one reciprocal at the end of each row
normalizes both outputs.
"""

import sys

for _p in ("/opt/trn_rl_repo", "/root/.axon_site"):
    if _p not in sys.path:
        sys.path.insert(0, _p)

import numpy as np

import concourse.bass as bass
import concourse.mybir as mybir
from concourse import bacc
from concourse.bass import ds, ts
from concourse.bass_utils import run_bass_kernel_spmd
from concourse.masks import make_identity
from concourse.tile import TileContext

F32 = mybir.dt.float32
BF16 = mybir.dt.bfloat16
I32 = mybir.dt.int32
AF = mybir.ActivationFunctionType
ALU = mybir.AluOpType

N_CORES = 8
B, S, ENC_D, DEC_D, ATTN_D, OUT_D = 32, 2048, 1024, 1024, 1024, 1024
BL = B // N_CORES          # batch rows per core
P = 128
EC = ENC_D // P            # enc-dim chunks (contraction)
AT = ATTN_D // P           # attn-dim tiles
OT = OUT_D // P            # out-dim tiles
ST = S // P                # sequence tiles of 128
NW = 4                     # windows per batch row
WST = ST // NW             # sequence tiles per window (4)
WS = WST * P               # window size in sequence positions (512)


def build_kernel():
    nc = bacc.Bacc("TRN2", target_bir_lowering=False, debug=False,
                   num_devices=N_CORES)

    enc = nc.dram_tensor("enc", [BL, S, ENC_D], F32, kind="ExternalInput").ap()
    dec = nc.dram_tensor("dec", [BL, DEC_D], F32, kind="ExternalInput").ap()
    elen = nc.dram_tensor("elen", [1, BL], I32, kind="ExternalInput").ap()
    W_enc = nc.dram_tensor("W_enc", [ENC_D, ATTN_D], F32, kind="ExternalInput").ap()
    b_enc = nc.dram_tensor("b_enc", [ATTN_D], F32, kind="ExternalInput").ap()
    W_dec = nc.dram_tensor("W_dec", [DEC_D, ATTN_D], F32, kind="ExternalInput").ap()
    b_dec = nc.dram_tensor("b_dec", [ATTN_D], F32, kind="ExternalInput").ap()
    w_attn = nc.dram_tensor("w_attn", [ATTN_D], F32, kind="ExternalInput").ap()
    W_out = nc.dram_tensor("W_out", [ENC_D, OUT_D], F32, kind="ExternalInput").ap()
    b_out = nc.dram_tensor("b_out", [OUT_D], F32, kind="ExternalInput").ap()
    ctx_out = nc.dram_tensor("ctx_out", [BL, OUT_D], F32, kind="ExternalOutput").ap()
    attn_out = nc.dram_tensor("attn_out", [BL, S], F32, kind="ExternalOutput").ap()

    with TileContext(nc) as tc:
        with (
            tc.tile_pool(name="wpool", bufs=1) as wpool,
            tc.tile_pool(name="wtmp", bufs=3) as wtmp,
            tc.tile_pool(name="const", bufs=1) as constp,
            tc.tile_pool(name="nat", bufs=6) as natp,
            tc.tile_pool(name="trp", bufs=5) as trp,
            tc.tile_pool(name="tanh", bufs=3) as tanhp,
            tc.tile_pool(name="small", bufs=2) as smallp,
            tc.tile_pool(name="ph", bufs=3, space="PSUM") as psh,
            tc.tile_pool(name="psc", bufs=2, space="PSUM") as pssc,
            tc.tile_pool(name="pctx", bufs=2, space="PSUM") as psctx,
            tc.tile_pool(name="pmisc", bufs=1, space="PSUM") as psmisc,
        ):
            # ---- first enc window starts streaming immediately (SWDGE queue
            # carries only these big cast-loads) ----
            nats = {}
            trs = {}

            def load_window(b, w):
                enc_b = enc[b].rearrange("(st p) e -> p st e", p=P)
                nat = natp.tile([P, WST, ENC_D], BF16, tag="nat")
                nc.gpsimd.dma_start(nat[:], enc_b[:, ds(w * WST, WST), :])
                tr = trp.tile([P, WST, EC, P], BF16, tag="encT")
                for st in range(WST):
                    nc.sync.dma_start_transpose(tr[:, st], nat[:, st, :])
                nats[(b, w)] = nat
                trs[(b, w)] = tr

            load_window(0, 0)

            # ---- small constants on the scalar HWDGE ring ----
            benc_sb = constp.tile([P, AT], F32, tag="benc")
            nc.scalar.dma_start(benc_sb[:], b_enc.rearrange("(t p) -> p t", p=P))
            bdec_sb = constp.tile([P, AT], F32, tag="bdec")
            nc.scalar.dma_start(bdec_sb[:], b_dec.rearrange("(t p) -> p t", p=P))
            bsum_sb = constp.tile([P, AT], F32, tag="bsum")
            nc.vector.tensor_tensor(bsum_sb[:], benc_sb[:], bdec_sb[:], op=ALU.add)
            wattn_f = constp.tile([P, AT], F32, tag="wattn_f")
            nc.scalar.dma_start(wattn_f[:], w_attn.rearrange("(c p) -> p c", p=P))
            wattn_sb = constp.tile([P, AT], BF16, tag="wattn")
            nc.vector.tensor_copy(wattn_sb[:], wattn_f[:])
            dec_sb = constp.tile([BL, DEC_D], F32, tag="dec_sb")
            nc.scalar.dma_start(dec_sb[:], dec)
            dec_bf = constp.tile([BL, DEC_D], BF16, tag="dec_bf")
            nc.vector.tensor_copy(dec_bf[:], dec_sb[:])
            elen_i = constp.tile([1, BL], I32, tag="elen_i")
            nc.scalar.dma_start(elen_i[:], elen)
            elen_f = constp.tile([1, BL], F32, tag="elen_f")
            nc.vector.tensor_copy(elen_f[:], elen_i[:])

            iota_i = constp.tile([P, ST], I32, tag="iota_i")
            nc.gpsimd.iota(iota_i[:], pattern=[[P, ST]], base=0, channel_multiplier=1)
            iota_f = constp.tile([P, ST], F32, tag="iota_f")
            nc.vector.tensor_copy(iota_f[:], iota_i[:])

            ident = constp.tile([P, P], BF16, tag="ident")
            make_identity(nc, ident[:])
            ones_col = constp.tile([P, 1], F32, tag="ones_col")
            nc.vector.memset(ones_col[:], 1.0)
            ones_row = constp.tile([1, P], F32, tag="ones_row")
            nc.vector.memset(ones_row[:], 1.0)

            elen_ps = psmisc.tile([P, BL], F32, tag="misc")
            nc.tensor.matmul(elen_ps[:], ones_row[:], elen_f[:], start=True, stop=True)
            len_bc = constp.tile([P, BL], F32, tag="len_bc")
            nc.vector.tensor_copy(len_bc[:], elen_ps[:])

            # ---- weights. Wdec goes over the scalar HWDGE ring (fp32) with
            # VectorE casting to bf16 so cbias is ready early; Wenc streams as
            # per-a-tile SWDGE cast-DMAs right behind the first window so the
            # first matmul can start ~9us in; Wout is deferred (needed last).
            Wdec_sb = wpool.tile([P, EC, ATTN_D], BF16, tag="Wdec")
            Wdec3 = W_dec.rearrange("(c p) a -> p c a", p=P)
            for t in range(AT):
                tmp = wtmp.tile([P, EC, P], F32, tag="wtmp")
                nc.scalar.dma_start(tmp[:], Wdec3[:, :, ts(t, P)])
                nc.vector.tensor_copy(Wdec_sb[:, :, ts(t, P)], tmp[:])

            Wenc_sb = wpool.tile([P, EC, ATTN_D], BF16, tag="Wenc")
            Wenc3 = W_enc.rearrange("(c p) a -> p c a", p=P)
            for t in range(AT):
                nc.gpsimd.dma_start(Wenc_sb[:, :, ts(t, P)], Wenc3[:, :, ts(t, P)])
            load_window(0, 1)

            # ---- dec path: combined per-(a,b) tanh bias = dec@W_dec+b_dec+b_enc
            decT = constp.tile([P, EC, BL], BF16, tag="decT")
            for c in range(EC):
                tp = psmisc.tile([P, BL], BF16, tag="misc")
                nc.tensor.transpose(tp[:], dec_bf[:, ts(c, P)], ident[0:BL, 0:BL])
                nc.vector.tensor_copy(decT[:, c, :], tp[:])
            cbias = constp.tile([P, AT, BL], F32, tag="cbias")
            for t in range(AT):
                hp = psmisc.tile([P, BL], F32, tag="misc")
                for c in range(EC):
                    nc.tensor.matmul(hp[:], Wdec_sb[:, c, ts(t, P)], decT[:, c, :],
                                     start=(c == 0), stop=(c == EC - 1))
                nc.scalar.activation(cbias[:, t, :], hp[:], AF.Identity,
                                     bias=bsum_sb[:, t:t + 1], scale=1.0)

            load_window(0, 2)
            Wout_sb = wpool.tile([P, EC, OUT_D], BF16, tag="Wout")
            bout_sb = constp.tile([P, OT], F32, tag="bout")
            nc.scalar.dma_start(bout_sb[:], b_out.rearrange("(t p) -> p t", p=P))

            ctxT_all = constp.tile([P, EC, BL], BF16, tag="ctxT_all")

            # ---- main loop over local batch rows ----
            for b in range(BL):
                scores_ps = pssc.tile([P, ST], F32, tag="scores")
                expm = smallp.tile([P, ST], F32, tag="expm")
                expm_bf = smallp.tile([P, ST], BF16, tag="expm_bf")
                ctx_acc = smallp.tile([P, EC], F32, tag="ctx_acc")

                for w in range(NW):
                    if (b, w) not in nats:
                        load_window(b, w)
                    # prefetch two windows ahead
                    gi = b * NW + w
                    for gj in (gi + 1, gi + 2):
                        nb, nw = divmod(gj, NW)
                        if nb < BL and (nb, nw) not in nats:
                            load_window(nb, nw)
                    if b == BL - 1 and w == 2:
                        # W_out is only needed by the tail projection; queue its
                        # cast-load behind the last enc windows
                        nc.gpsimd.dma_start(
                            Wout_sb[:], W_out.rearrange("(c p) a -> p c a", p=P))

                    nat = nats[(b, w)]
                    tr = trs[(b, w)]

                    th = tanhp.tile([P, AT, WS], BF16, tag="tanh")
                    for t in range(AT):
                        hp = psh.tile([P, WS], F32, tag="hmain")
                        for c in range(EC):
                            nc.tensor.matmul(hp[:], Wenc_sb[:, c, ts(t, P)],
                                             tr[:, :, c, :],
                                             start=(c == 0), stop=(c == EC - 1))
                        nc.scalar.activation(th[:, t, :], hp[:], AF.Tanh,
                                             bias=cbias[:, t, b:b + 1], scale=1.0)

                    # scores for the 4 sequence tiles of this window
                    for j in range(WST):
                        sj = w * WST + j
                        for t in range(AT):
                            nc.tensor.matmul(scores_ps[:, sj:sj + 1],
                                             th[:, t, ts(j, P)],
                                             wattn_sb[:, t:t + 1],
                                             start=(t == 0), stop=(t == AT - 1))

                    # masked exp for this window (no max-sub: scores bounded)
                    ex = smallp.tile([P, WST], F32, tag="ex")
                    nc.scalar.activation(ex[:], scores_ps[:, ds(w * WST, WST)], AF.Exp)
                    va = smallp.tile([P, WST], F32, tag="va")
                    nc.vector.tensor_scalar(va[:], iota_f[:, ds(w * WST, WST)],
                                            len_bc[:, b:b + 1], None, op0=ALU.is_lt)
                    nc.vector.tensor_tensor(expm[:, ds(w * WST, WST)], ex[:], va[:],
                                            op=ALU.mult)
                    nc.vector.tensor_copy(expm_bf[:, ds(w * WST, WST)],
                                          expm[:, ds(w * WST, WST)])

                    # unnormalized context partial: enc_nat^T @ expm
                    cp = psctx.tile([P, EC], F32, tag="cp")
                    for t in range(EC):
                        for st in range(WST):
                            nc.tensor.matmul(cp[:, t:t + 1], nat[:, st, ts(t, P)],
                                             expm_bf[:, w * WST + st:w * WST + st + 1],
                                             start=(st == 0), stop=(st == WST - 1))
                    if w == 0:
                        nc.vector.tensor_copy(ctx_acc[:], cp[:])
                    else:
                        nc.vector.tensor_tensor(ctx_acc[:], ctx_acc[:], cp[:],
                                                op=ALU.add)
                    del nats[(b, w)], trs[(b, w)]

                # softmax denominator: cross-partition sum via PE ones-matmul
                sums = smallp.tile([P, 1], F32, tag="sums")
                nc.vector.tensor_reduce(sums[:], expm[:], axis=mybir.AxisListType.X,
                                        op=ALU.add)
                tot_ps = psmisc.tile([1, 1], F32, tag="misc")
                nc.tensor.matmul(tot_ps[:], ones_col[:], sums[:], start=True, stop=True)
                recip = smallp.tile([1, 1], F32, tag="recip")
                nc.vector.reciprocal(recip[:], tot_ps[:])
                rb_ps = psmisc.tile([P, 1], F32, tag="misc")
                nc.tensor.matmul(rb_ps[:], ones_row[:], recip[:], start=True, stop=True)
                rb = smallp.tile([P, 1], F32, tag="rb")
                nc.vector.tensor_copy(rb[:], rb_ps[:])

                attn_f = smallp.tile([P, ST], F32, tag="attn_f")
                nc.vector.tensor_scalar(attn_f[:], expm[:], rb[:], None, op0=ALU.mult)
                nc.gpsimd.dma_start(attn_out[b].rearrange("(j p) -> p j", p=P),
                                    attn_f[:])
                nc.vector.tensor_scalar(ctxT_all[:, :, b], ctx_acc[:], rb[:], None,
                                        op0=ALU.mult)

            # ---- final projection: ctx @ W_out + b_out ----
            out_sb = constp.tile([P, OT, BL], F32, tag="out_sb")
            for t in range(OT):
                op_ps = psmisc.tile([P, BL], F32, tag="misc")
                for c in range(EC):
                    nc.tensor.matmul(op_ps[:], Wout_sb[:, c, ts(t, P)],
                                     ctxT_all[:, c, :],
                                     start=(c == 0), stop=(c == EC - 1))
                nc.scalar.activation(out_sb[:, t, :], op_ps[:], AF.Identity,
                                     bias=bout_sb[:, t:t + 1], scale=1.0)
            for b in range(BL):
                nc.gpsimd.dma_start(ctx_out[b].rearrange("(t p) -> p t", p=P),
                                    out_sb[:, :, b])

    nc.compile()
    return nc


_NC_CACHE = {}


def _get_nc():
    if "nc" not in _NC_CACHE:
        _NC_CACHE["nc"] = build_kernel()
    return _NC_CACHE["nc"]


def kernel(enc_states, dec_states, W_enc, b_enc, W_dec, b_dec, w_attn, W_out,
           b_out, enc_len, _trace=False, _trace_kwargs=None):
    nc = _get_nc()

    enc_states = np.ascontiguousarray(np.asarray(enc_states, dtype=np.float32))
    dec_states = np.ascontiguousarray(np.asarray(dec_states, dtype=np.float32))
    enc_len_i = np.asarray(enc_len).astype(np.int32)
    shared = {
        "W_enc": np.ascontiguousarray(np.asarray(W_enc, dtype=np.float32)),
        "b_enc": np.ascontiguousarray(np.asarray(b_enc, dtype=np.float32)),
        "W_dec": np.ascontiguousarray(np.asarray(W_dec, dtype=np.float32)),
        "b_dec": np.ascontiguousarray(np.asarray(b_dec, dtype=np.float32)),
        "w_attn": np.ascontiguousarray(np.asarray(w_attn, dtype=np.float32)),
        "W_out": np.ascontiguousarray(np.asarray(W_out, dtype=np.float32)),
        "b_out": np.ascontiguousarray(np.asarray(b_out, dtype=np.float32)),
    }
    in_maps = []
    for i in range(N_CORES):
        sl = slice(i * BL, (i + 1) * BL)
        in_maps.append({
            "enc": enc_states[sl],
            "dec": dec_states[sl],
            "elen": enc_len_i[sl].reshape(1, BL),
            **shared,
        })

    res = run_bass_kernel_spmd(nc, in_maps, core_ids=list(range(N_CORES)),
                               trace=_trace, **(_trace_kwargs or {}))

    context = np.concatenate([r["ctx_out"] for r in res.results], axis=0)
    attn = np.concatenate([r["attn_out"] for r in res.results], axis=0)
    if _trace:
        return (context, attn), res
    return (context, attn)
